# revision 29
# baseline (speedup 1.0000x reference)
"""KAN layer (uniform cubic B-spline, grid=8, k=3) Trainium2 kernel.

Math
----
Reference computes, per batch row n and output o:
    out[n,o] = sum_i w_silu[i,o]*silu(x[n,i]) + sum_i w_sp[i,o] * sum_b B_b(x[n,i]) * C[b,i,o]

With the uniform knot grid t_j = -1.75 + 0.25*j, put s = 4x+7 in [3,11). The
normalized cubic B-spline has the truncated-power form
    B_j(s) = sum_p w5[p] * (s-(j+p))_+^3 / 6,   w5 = [1,-4,6,-4,1].
Naive truncated-power tiles are numerically catastrophic (values ~222 cancel
to O(1)). Split each knot term two-sidedly: mirrored cubes L_k=(k-s)_+^3/6 for
k in {4,5,6} (small since s>=3), direct cubes R_k=(s-k)_+^3/6 for k in {7..10}
(small since s<11); k<=3 terms are globally polynomial, k>=11 terms vanish.
The leftover per-j cubic polynomial is expanded in centered monomials
{1, x, x^2, x^3} (all bounded by 1). The constant (x^0) component does not
depend on the batch row, so it is folded into a per-output bias on the host.
Everything else folds into 11 weight groups of shape (n_in, n_out):
    [P1=x, P2=x^2, P3=x^3, silu, L4, L5, L6, R7, R8, R9, R10]
so the device does: 10 cheap elementwise activation maps + one fp16 matmul
with contraction dim 11*512, accumulated in fp32 PSUM, plus a bias add on the
PSUM evacuation. Everything is fp16 end-to-end, including the x input itself
(so the raw input tile IS the P1 activation group): fp16's 11-bit mantissa
keeps the total error at ~6.7e-3 rel L2 (measured against the float64
reference on the real input distribution) while halving DMA vs fp32r; the
2e-2 gate has 3x margin. bf16 (~1.8e-2) and fp8 (catastrophic, the
decomposition cancels heavily) were measured and rejected.

Schedule engineering (the PE stream runs at the 216 ns/matmul roofline):
 - a warm-up burst of matmuls on a memset tile bridges the DMA lead-in so the
   PE's HAM clock-gate reaches 8/8 before the real stream starts;
 - x chunks go on the scalar HWDGE queue concurrently with weights on sync;
 - the first weight group arrives in per-g 128 KB chunks and the first matmul
   group is ordered g-outer, so the stream starts after ~256 KB of DMA; the
   bias transfer is deferred past the lead-in burst (HBM contention with all
   8 cores fetching at once makes early-DMA latency ~3-5 us);
 - the last two groups are ordered m-major so the four PSUM bank chains end
   staggered ~1.7 us apart and evacuation/output-DMA overlap the stream tail;
 - the output is written to HBM as fp16 and upcast on the host.

Sharding: data-parallel over the batch axis N across 8 cores (512 rows each);
weights replicated. No collectives.
"""

import numpy as np

N, N_IN, N_OUT = 4096, 512, 512
NB = 11
NCORES = 8
ROWS = N // NCORES          # batch rows per core
G = N_IN // 128             # 4 partition groups over n_in
M = ROWS // 128             # 4 PSUM row-chunks
W5 = (1.0, -4.0, 6.0, -4.0, 1.0)
NWARM = 6                   # HAM warm-up matmuls during the DMA lead-in

# cube groups: (kind, knot, path)
#   path "a": T1=ACT relu(affine), T2=ACT square(T1), tile=DVE T1*T2
#   path "e": T1=DVE ts(U -k, max/min 0), T2=ACT square(affine), tile=DVE T1*T2
# for ("L", k, "e") the DVE min-trick yields -(k-s)_+^3/6, so weight flips sign.
CUBES = [
    ("L", 4, "e"), ("L", 5, "e"), ("L", 6, "a"),
    ("R", 7, "a"), ("R", 8, "a"), ("R", 9, "e"), ("R", 10, "e"),
]
NGROUPS = 4 + len(CUBES)    # [P1, P2, P3, silu, cubes...]

_CACHE = {}


def _poly_alpha():
    """alpha[j, t]: coefficient of x^t in the polynomial part of B_j."""
    alpha = np.zeros((NB, 4), dtype=np.float64)
    for j in range(NB):
        for p in range(5):
            k = j + p
            if k <= 6:  # (s-k)^3/6 with s-k = 4x + (7-k)
                a = 7.0 - k
                alpha[j, 3] += W5[p] * 64.0 / 6.0
                alpha[j, 2] += W5[p] * 48.0 * a / 6.0
                alpha[j, 1] += W5[p] * 12.0 * a * a / 6.0
                alpha[j, 0] += W5[p] * a * a * a / 6.0
    return alpha


def _prep_weights(C, w_silu, w_sp):
    """Fold C*w_sp through the decomposition into 11 fp16 weight groups,
    ordered [P1, P2, P3, silu, cubes...], plus the fp32 bias row (the
    batch-independent x^0 component). float64 internally."""
    Ceff = C.astype(np.float64) * w_sp.astype(np.float64)[None]
    alpha = _poly_alpha()
    beta = np.einsum("jt,jio->tio", alpha, Ceff)  # (4, n_in, n_out)
    Wt = np.empty((NGROUPS, N_IN, N_OUT), dtype=np.float64)
    Wt[0] = beta[1]
    Wt[1] = beta[2]
    Wt[2] = beta[3]
    Wt[3] = w_silu.astype(np.float64)
    for gi, (kind, k, path) in enumerate(CUBES):
        wk = np.zeros((N_IN, N_OUT), dtype=np.float64)
        for p in range(5):
            j = k - p
            if 0 <= j < NB:
                wk += W5[p] * Ceff[j]
        if kind == "L" and path == "e":
            wk = -wk
        Wt[4 + gi] = wk
    bias = beta[0].sum(axis=0)  # (n_out,) batch-independent component
    bT = np.ascontiguousarray(
        np.broadcast_to(bias.astype(np.float32)[None, :], (128, N_OUT))
    )
    return Wt.astype(np.float16), bT


def _build():
    import concourse.bacc as bacc
    import concourse.mybir as mybir
    from concourse import tile

    f32 = mybir.dt.float32
    f16 = mybir.dt.float16
    AF = mybir.ActivationFunctionType
    ALU = mybir.AluOpType

    c3 = 6.0 ** (-1.0 / 3.0)   # cube-root scaling for path "a"
    c2 = 6.0 ** (-0.5)         # sqrt scaling for path "e" squares

    nc = bacc.Bacc("TRN2", target_bir_lowering=False, debug=False)
    XT = nc.dram_tensor("xT", [N_IN, ROWS], f16, kind="ExternalInput").ap()
    WT = nc.dram_tensor("Wt", [NGROUPS, N_IN, N_OUT], f16, kind="ExternalInput").ap()
    BT = nc.dram_tensor("bT", [128, N_OUT], f32, kind="ExternalInput").ap()
    OUT = nc.dram_tensor("out", [ROWS, N_OUT], f16, kind="ExternalOutput").ap()

    with tile.TileContext(nc) as tc:
        with (
            tc.tile_pool(name="const", bufs=1) as constp,
            tc.tile_pool(name="dqp", bufs=5) as dqp,
            tc.tile_pool(name="t1p", bufs=3) as t1p,
            tc.tile_pool(name="t2p", bufs=3) as t2p,
            tc.tile_pool(name="cubep", bufs=3) as cubep,
            tc.tile_pool(name="outp", bufs=4) as outp,
            tc.tile_pool(name="psp", bufs=1, space="PSUM") as psp,
        ):
            # ---- HAM warm-up: PE busy on a DVE-memset tile during the DMA
            # lead-in so the clock-gate reaches 8/8 before the real stream.
            wm = constp.tile([128, 640], f16)
            nc.vector.memset(wm[:], 0.0)
            wps = psp.tile([128, 512], f32, name="wps", tag="wps")
            for i in range(NWARM):
                nc.tensor.matmul(
                    wps[:], wm[:, 0:128], wm[:, 128:640],
                    start=(i == 0), stop=(i == NWARM - 1),
                )

            # x arrives AS FP16 (512 KB) in two half-tiles on the scalar HWDGE
            # queue, concurrent with the weight DMAs on the sync queue. The
            # raw tile IS the first matmul group's activation (P1 = x), so the
            # stream starts as soon as the first x half + first weight chunk
            # land. Bias is only needed at evacuation time -> issued last.
            xt = constp.tile([128, G, ROWS], f16)
            XTr = XT.rearrange("(g p) n -> p g n", p=128)
            H01, H23 = slice(0, 2), slice(2, 4)
            # g=0's x rows split once more so the very first matmul (m=0) is
            # gated on a 64 KB transfer instead of 128 KB
            nc.scalar.dma_start(xt[:, 0:1, 0:256], XTr[:, 0:1, 0:256])
            nc.scalar.dma_start(xt[:, 0:1, 256:512], XTr[:, 0:1, 256:512])
            for g in range(1, G):
                nc.scalar.dma_start(xt[:, g : g + 1, :], XTr[:, g : g + 1, :])
            bias = constp.tile([128, N_OUT], f32)

            p1 = xt
            p2 = constp.tile([128, G, ROWS], f16)
            p3 = constp.tile([128, G, ROWS], f16)
            for h in (H01, H23):
                nc.vector.tensor_tensor(p2[:, h, :], xt[:, h, :], xt[:, h, :], op=ALU.mult)
                nc.vector.tensor_tensor(p3[:, h, :], p2[:, h, :], xt[:, h, :], op=ALU.mult)

            # silu first among ACT ops so its table set (with the square/relu
            # fillers) loads exactly once.
            sil = constp.tile([128, G, ROWS], f16)
            nc.scalar.activation(sil[:], xt[:], AF.Silu)

            # U = s = 4x + 7 (feeds the "e"-path clamps)
            U = constp.tile([128, G, ROWS], f16)
            nc.vector.tensor_scalar(U[:], xt[:], 4.0, 7.0, op0=ALU.mult, op1=ALU.add)

            # bias constants for the ACT affine maps, one column per cube
            bias_a = constp.tile([128, len(CUBES)], f32)
            bias_e = constp.tile([128, len(CUBES)], f32)
            for gi, (kind, k, path) in enumerate(CUBES):
                sgn = -1.0 if kind == "L" else 1.0
                nc.gpsimd.memset(bias_a[:, gi : gi + 1], sgn * (7.0 - k) * c3)
                nc.gpsimd.memset(bias_e[:, gi : gi + 1], (7.0 - k) * c2)

            psums = [
                psp.tile([128, N_OUT], f32, name=f"ps{m}", tag=f"ps{m}") for m in range(M)
            ]

            def emit_matmuls(gidx, act, dq):
                # g-outer so early groups only need the xt chunk that has landed
                first = gidx == 0
                for g in range(G):
                    for m in range(M):
                        if first and g == 0 and m == 0:
                            # split the stream's opening matmul into two N=256
                            # halves: the first is gated on only 64 KB of
                            # weights. start=True clears the whole bank, the
                            # second half overwrites its (cleared) region.
                            nc.tensor.matmul(
                                psums[0][:, 0:256],
                                act[:, 0, 0:128],
                                dq[:, 0, 0:256],
                                start=True,
                                stop=False,
                            )
                            nc.tensor.matmul(
                                psums[0][:, 256:512],
                                act[:, 0, 0:128],
                                dq[:, 0, 256:512],
                                start=False,
                                stop=False,
                            )
                            continue
                        nc.tensor.matmul(
                            psums[m][:],
                            act[:, g, m * 128 : (m + 1) * 128],
                            dq[:, g, :],
                            start=(first and g == 0),
                            stop=False,
                        )

            acts = {}
            dqs = {}
            for gidx in range(NGROUPS):
                dq = dqp.tile([128, G, N_OUT], f16)
                WTr = WT[gidx].rearrange("(g p) o -> p g o", p=128)
                if gidx == 0:
                    # first group's weights in per-g chunks (g=0 split again to
                    # 64 KB): the stream's opening N=256 matmul starts after
                    # just 128 KB of total DMA
                    nc.sync.dma_start(dq[:, 0:1, 0:256], WTr[:, 0:1, 0:256])
                    nc.sync.dma_start(dq[:, 0:1, 256:512], WTr[:, 0:1, 256:512])
                    for g in range(1, G):
                        nc.sync.dma_start(dq[:, g : g + 1, :], WTr[:, g : g + 1, :])
                else:
                    nc.sync.dma_start(dq[:], WTr)
                if gidx == 2:
                    # bias is only read at evacuation (~45us in); issue it here
                    # so its transfer doesn't steal lead-in HBM bandwidth
                    nc.sync.dma_start(bias[:], BT)
                if gidx == 0:
                    act = p1
                elif gidx == 1:
                    act = p2
                elif gidx == 2:
                    act = p3
                elif gidx == 3:
                    act = sil
                else:
                    ci = gidx - 4
                    kind, k, path = CUBES[ci]
                    cube = cubep.tile([128, G, ROWS], f16, name="cube", tag="cube")
                    if path == "a":
                        scale = (-4.0 if kind == "L" else 4.0) * c3
                        t1 = t1p.tile([128, G, ROWS], f16, name="t1", tag="t1")
                        nc.scalar.activation(
                            t1[:], xt[:], AF.Relu, bias=bias_a[:, ci : ci + 1], scale=scale
                        )
                        t2 = t2p.tile([128, G, ROWS], f16, name="t2", tag="t2")
                        nc.scalar.activation(t2[:], t1[:], AF.Square)
                        nc.vector.tensor_tensor(cube[:], t1[:], t2[:], op=ALU.mult)
                    else:
                        # T1 = (s-k) clamped toward zero from the correct side
                        clamp = ALU.min if kind == "L" else ALU.max
                        t1 = t1p.tile([128, G, ROWS], f16, name="t1", tag="t1")
                        nc.vector.tensor_scalar(
                            t1[:], U[:], float(k), 0.0, op0=ALU.subtract, op1=clamp
                        )
                        t2 = t2p.tile([128, G, ROWS], f16, name="t2", tag="t2")
                        nc.scalar.activation(
                            t2[:], xt[:], AF.Square, bias=bias_e[:, ci : ci + 1], scale=4.0 * c2
                        )
                        nc.vector.tensor_tensor(cube[:], t1[:], t2[:], op=ALU.mult)
                    act = cube
                if gidx < NGROUPS - 2:
                    emit_matmuls(gidx, act, dq)
                else:
                    acts[gidx] = act
                    dqs[gidx] = dq

            # Last two groups m-major so the four PSUM bank chains finish
            # staggered ~1.7us apart; each bank's evacuation + output DMA then
            # overlaps the remaining matmuls instead of queueing at the end.
            for m in range(M):
                for gidx in (NGROUPS - 2, NGROUPS - 1):
                    for g in range(G):
                        nc.tensor.matmul(
                            psums[m][:],
                            acts[gidx][:, g, m * 128 : (m + 1) * 128],
                            dqs[gidx][:, g, :],
                            start=False,
                            stop=(gidx == NGROUPS - 1 and g == G - 1),
                        )
                ot = outp.tile([128, N_OUT], f16, name="ot", tag="ot")
                nc.vector.tensor_tensor(ot[:], psums[m][:], bias[:], op=ALU.add)
                nc.sync.dma_start(OUT[m * 128 : (m + 1) * 128, :], ot[:])

    nc.compile()
    return nc


# test-harness knobs (the grader just calls kernel())
TRACE = False
LAST_RESULTS = None


def kernel(x, grid, C, w_silu, w_sp):
    from concourse import bass_utils

    if "nc" not in _CACHE:
        _CACHE["nc"] = _build()
    nc = _CACHE["nc"]

    x = np.asarray(x, dtype=np.float32).astype(np.float16)
    Wt, bT = _prep_weights(np.asarray(C), np.asarray(w_silu), np.asarray(w_sp))

    in_maps = []
    for c in range(NCORES):
        xT = np.ascontiguousarray(x[c * ROWS : (c + 1) * ROWS].T)
        in_maps.append({"xT": xT, "Wt": Wt, "bT": bT})

    res = bass_utils.run_bass_kernel_spmd(
        nc, in_maps, core_ids=list(range(NCORES)), trace=TRACE
    )
    global LAST_RESULTS
    LAST_RESULTS = res
    return np.concatenate(
        [res.results[c]["out"].astype(np.float32) for c in range(NCORES)], axis=0
    )
